# revision 1
# baseline (speedup 1.0000x reference)
"""BinaryLinear Trainium2 kernel.

Computes: out = binarize(x) @ binarize(weight - threshold).T * 2^round(clip(shift, -8, 0))

where binarize(v) = +1 if v >= 0 else -1, over x [B,S,IN], weight [OUT,IN].

Strategy (8 NeuronCores, tensor-parallel over OUT):
  - each core gets the full x and a 2048-row slice of weight/threshold
  - host prep (lossless for the computation): cast x/w to bf16 -- the
    device only uses the sign, and bf16 round-to-nearest preserves the
    sign of every value of these distributions (no magnitude reaches the
    bf16 flush range; exact +/-0 both binarize to +1 via is_ge) -- plus
    a column interleave and transpose of w (layout only, see below)
  - on device: binarize to +/-0.5 (one fused DVE op, exact in fp8e4m3);
    the missing x4 is folded into the final output scale
  - fp8 DoubleRow matmuls (256 contraction rows per matmul, 2x PE rate)
    accumulate into fp32 PSUM; weights are the stationary operand (its
    DoubleRow pair-dim must be 16B-aligned -> grouped k-tile layout,
    loaded directly from the host-transposed wT), x is the moving
    operand (pairs may be byte-adjacent -> packed layout produced by the
    2-byte hardware DMA-transpose of fp8 pairs inside bf16 elements)
  - the grouped-vs-packed pair mapping is reconciled by the host column
    interleave of w; the [OUT, S] device output is transposed back on
    the host during the gather
  - queue discipline: sync HWDGE = loads only, scalar HWDGE =
    DMA-transposes only, DVE = binarize/psum-evict, gpsimd SWDGE =
    output stores (keeps psum eviction off queues with long-wait heads)
  - result is bit-exact (all products +/-0.25, exact fp32 accumulation)
"""

import sys

if "/opt/trn_rl_repo" not in sys.path:
    sys.path.insert(0, "/opt/trn_rl_repo")

import numpy as np

B, S, IN, OUT = 4, 2048, 4096, 16384
N_CORES = 8
O_SHARD = OUT // N_CORES  # 2048
P = 128  # partitions
N_CH = 512  # psum free-dim chunk (one bank of fp32)

# dev knobs (test.py only; harness uses defaults)
_TRACE = False
_LAST_RESULTS = None


def build_program(s_rows=B * S, o_shard=O_SHARD, kdim=IN, scale=1.0,
                  zero_thr=True):
    """Trace the single-core SPMD program.

    Inputs: x [s_rows,kdim] bf16, w [kdim,o_shard] bf16 (host-interleaved
    columns then transposed), thr [o_shard,1] f32.
    Output: outT [o_shard,s_rows] f32.
    """
    import concourse.bass as bass
    import concourse.mybir as mybir
    import concourse.tile as tile
    from concourse import bacc
    from concourse.alu_op_type import AluOpType

    f32 = mybir.dt.float32
    bf16 = mybir.dt.bfloat16
    fp8 = mybir.dt.float8e4

    n_g = kdim // 256      # DoubleRow groups (256 contraction rows each)
    n_kt = kdim // P       # 128-row k-tiles in the stationary slab
    n_ob = o_shard // P    # o-blocks of 128
    n_pass = n_ob // 4     # 4 o-blocks (psum banks) per pass
    n_sc = s_rows // N_CH  # s-chunks of 512
    MC = min(8, n_kt)      # k-tiles per w load chunk
    n_mc = n_kt // MC
    assert s_rows % N_CH == 0 and o_shard % (4 * P) == 0 and kdim % 256 == 0
    assert n_kt % MC == 0

    nc = bacc.Bacc(None, target_bir_lowering=False, debug=False)

    w_dt = bf16 if zero_thr else f32
    x_d = nc.dram_tensor("x", [s_rows, kdim], bf16, kind="ExternalInput")
    w_d = nc.dram_tensor("w", [kdim, o_shard], w_dt, kind="ExternalInput")
    t_d = nc.dram_tensor("thr", [o_shard, 1], f32, kind="ExternalInput")
    o_d = nc.dram_tensor("outT", [o_shard, s_rows], f32, kind="ExternalOutput")

    with tile.TileContext(nc) as tc:
        with (
            tc.tile_pool(name="raw", bufs=3) as raw_pool,
            tc.tile_pool(name="wld", bufs=2) as wld_pool,
            tc.tile_pool(name="b8", bufs=5) as b8_pool,
            tc.tile_pool(name="xt", bufs=3) as xt_pool,
            tc.tile_pool(name="w8", bufs=1) as w8_pool,
            tc.tile_pool(name="outp", bufs=3) as out_pool,
            tc.tile_pool(name="misc", bufs=1) as misc_pool,
            tc.tile_pool(name="ps", bufs=2, space="PSUM") as ps_pool,
        ):
            # Binarized values live as fp8 (+/-0.5). Two fp8 values for
            # consecutive (interleaved) contraction rows pack into one
            # bf16-typed element so the 2-byte hardware DMA-transpose moves
            # them in one shot; the pair becomes DoubleRow's two k-groups
            # via a bitcast AP.

            wslabs = [
                w8_pool.tile([P, n_kt, 4 * P], fp8, name=f"wslab{ps}",
                             tag=f"wslab{ps}")
                for ps in range(n_pass)
            ]

            thr_rep = None
            if not zero_thr:
                # broadcast thr [o_shard] across partitions via a rank-1
                # matmul: ones[1,128].T @ thr_row[1, o] -> [128, o]
                thr_rep = misc_pool.tile([P, o_shard], f32, name="thr_rep")
                ones_t = misc_pool.tile([P, P], f32, name="ones_t")
                thr_row = misc_pool.tile([P, o_shard], f32, name="thr_row")
                nc.vector.memset(ones_t[:], 1.0)
                nc.sync.dma_start(thr_row[:1, :],
                                  t_d[:, :].rearrange("o one -> one o"))
                for q in range(o_shard // N_CH):
                    tps = ps_pool.tile([P, N_CH], f32, name="tps", tag="ps0")
                    nc.tensor.matmul(tps[:], ones_t[:1, :P],
                                     thr_row[:1, q * N_CH:(q + 1) * N_CH],
                                     start=True, stop=True)
                    nc.vector.tensor_copy(
                        thr_rep[:, q * N_CH:(q + 1) * N_CH], tps[:])

            def prep_chunk(ps, mc, eng=None):
                # one DMA pulls MC k-tiles of this pass's o-range into
                # [p, t, o] layout straight from the host-transposed wT
                wtile = wld_pool.tile([P, MC, 4 * P], w_dt, name="wtile",
                                      tag="wld")
                src = w_d[mc * MC * P:(mc + 1) * MC * P,
                          ps * 4 * P:(ps + 1) * 4 * P]
                (eng or nc.sync).dma_start(
                    wtile[:], src.rearrange("(t p) o -> p t o", p=P))
                dst = wslabs[ps][:, mc * MC:(mc + 1) * MC, :]
                if zero_thr:
                    # (w >= 0) - 0.5 -> +/-0.5, exact in fp8e4m3
                    nc.vector.tensor_scalar(
                        dst, wtile[:], 0.0, 0.5,
                        AluOpType.is_ge, AluOpType.subtract)
                else:
                    for t in range(MC):
                        sel = thr_rep[:, ps * 4 * P:(ps + 1) * 4 * P]
                        nc.vector.scalar_tensor_tensor(
                            dst[:, t, :], wtile[:, t, :], 1.0, sel,
                            op0=AluOpType.mult, op1=AluOpType.is_ge)
                        nc.vector.tensor_scalar(
                            dst[:, t, :], dst[:, t, :], 0.5, None,
                            AluOpType.subtract)

            def emit_chain(sc):
                # x moving tile [p, g, 512 s] as packed fp8 pairs in bf16:
                # filled by 4 DMA-transposes (one per 128-row s-subblock)
                xtile = xt_pool.tile([P, n_g, N_CH], bf16, name="xtile",
                                     tag="xt")
                xb8s = []
                for sub in range(4):
                    s0 = sc * N_CH + sub * P
                    x_raw = raw_pool.tile([P, kdim], bf16, name="x_raw",
                                          tag="raw")
                    nc.sync.dma_start(x_raw[:], x_d[s0:s0 + P, :])
                    xb8 = b8_pool.tile([P, kdim], fp8, name="xb8", tag="b8")
                    nc.vector.tensor_scalar(
                        xb8[:], x_raw[:], 0.0, 0.5,
                        AluOpType.is_ge, AluOpType.subtract)
                    xb8s.append(xb8)
                for sub in range(4):
                    nc.scalar.dma_start(
                        xtile[:, :, sub * P:(sub + 1) * P],
                        xb8s[sub][:].bitcast(bf16), transpose=True)
                return xtile.bitcast(fp8)  # [p, g, 1024] (s,j interleaved)

            # prep pass slabs in demand order, first x chains interleaved;
            # later passes load over the idle SWDGE path so the sync queue
            # stays clear for the early x chains
            chains = {}
            for mc in range(n_mc):
                prep_chunk(0, mc)
            if n_sc >= 2:
                chains[0] = emit_chain(0)
            for ps in range(1, n_pass):
                eng = nc.gpsimd if ps >= 2 else None
                for mc in range(n_mc):
                    prep_chunk(ps, mc, eng)
                if ps == 1 and n_sc >= 2:
                    chains[1] = emit_chain(1)
            if n_sc >= 3:
                chains[2] = emit_chain(2)

            # --- main loop over s-chunks of 512 ---
            for sc in range(n_sc):
                xt8 = chains.pop(sc, None)
                if xt8 is None:
                    xt8 = emit_chain(sc)

                for ps in range(n_pass):
                    pss = [
                        ps_pool.tile([P, N_CH], f32, name=f"ps{i}",
                                     tag=f"ps{i}")
                        for i in range(4)
                    ]
                    for g in range(n_g):
                        rhs = xt8[:, g, :].rearrange("p (s j) -> p j s", j=2)
                        for i in range(4):
                            nc.tensor.matmul(
                                pss[i][:],
                                wslabs[ps][:, 2 * g:2 * g + 2,
                                           i * P:(i + 1) * P],
                                rhs,
                                start=(g == 0), stop=(g == n_g - 1),
                                perf_mode=mybir.MatmulPerfMode.DoubleRow)
                    for i in range(4):
                        ob = ps * 4 + i
                        ot = out_pool.tile([P, N_CH], f32, name="ot", tag="ot")
                        # psum eviction with the pow2 scale folded in
                        nc.vector.tensor_scalar(
                            ot[:], pss[i][:], float(scale), None,
                            AluOpType.mult)
                        nc.gpsimd.dma_start(
                            o_d[ob * P:(ob + 1) * P,
                                sc * N_CH:(sc + 1) * N_CH], ot[:])

    nc.compile()
    return nc


def _host_scale(shift_param):
    # 4x undoes the two 0.5 factors from binarizing to +/-0.5;
    # np.round is round-half-to-even, matching jnp.round.
    s = np.clip(np.float64(np.float32(shift_param)), -8.0, 0.0)
    return 4.0 * float(np.exp2(np.round(s)))


def _interleave_w_cols(w):
    """Host permutation so the device's grouped stationary layout pairs the
    same contraction rows as the packed moving layout: new col 256g+128j+p
    holds old col 256g+2p+j."""
    o, k = w.shape
    return np.ascontiguousarray(
        w.reshape(o, k // 256, 128, 2).transpose(0, 1, 3, 2).reshape(o, k))


def kernel(x, weight, threshold, shift_param):
    import ml_dtypes

    from concourse.bass_utils import run_bass_kernel_spmd

    bf16 = ml_dtypes.bfloat16
    scale = _host_scale(shift_param)
    thr_f = np.asarray(threshold, np.float32).reshape(OUT, 1)
    zero_thr = bool(np.all(thr_f == 0.0))
    nc = build_program(scale=scale, zero_thr=zero_thr)

    xf = np.ascontiguousarray(
        x.astype(np.float32).reshape(B * S, IN).astype(bf16))
    # layout-only host prep of w: column interleave, transpose; the cast
    # to bf16 is sign-exact and only taken when threshold is all-zero
    wt = _interleave_w_cols(weight.astype(np.float32))
    wt = (wt.astype(bf16) if zero_thr else wt).T
    in_maps = []
    for c in range(N_CORES):
        sl = slice(c * O_SHARD, (c + 1) * O_SHARD)
        in_maps.append({
            "x": xf,
            "w": np.ascontiguousarray(wt[:, sl]),
            "thr": np.ascontiguousarray(thr_f[sl]),
        })

    res = run_bass_kernel_spmd(nc, in_maps, list(range(N_CORES)), trace=_TRACE)
    global _LAST_RESULTS
    _LAST_RESULTS = res
    shards = [res.results[c]["outT"] for c in range(N_CORES)]
    full_t = np.concatenate(shards, axis=0)  # [OUT, B*S]
    full = np.ascontiguousarray(full_t.T).reshape(B, S, OUT)
    return full.astype(np.float32)



# revision 4
# speedup vs baseline: 1.0974x; 1.0974x over previous
"""BinaryLinear Trainium2 kernel.

Computes: out = binarize(x) @ binarize(weight - threshold).T * 2^round(clip(shift, -8, 0))

where binarize(v) = +1 if v >= 0 else -1, over x [B,S,IN], weight [OUT,IN].

Strategy (8 NeuronCores, tensor-parallel over OUT):
  - each core gets the full x and a 2048-row slice of weight/threshold
  - host prep (lossless for the computation): cast x/w to bf16 -- the
    device only uses the sign, and bf16 round-to-nearest preserves the
    sign of every value of these distributions (no magnitude reaches the
    bf16 flush range; exact +/-0 both binarize to +1 via is_ge) -- plus
    a column interleave and transpose of w (layout only, see below)
  - on device: x binarizes to +/-0.5 (one fused DVE op, exact in
    fp8e4m3); w binarizes to +/-1 on the Activation engine via
    Sign(w + 2^-140) -- the tiny bias makes Sign match is_ge exactly for
    every bf16 input; the product's missing x2 is folded into the final
    output scale
  - fp8 DoubleRow matmuls (256 contraction rows per matmul, 2x PE rate)
    accumulate into fp32 PSUM; weights are the stationary operand (its
    DoubleRow pair-dim must be 16B-aligned -> grouped k-tile layout,
    loaded directly from the host-transposed wT), x is the moving
    operand (pairs may be byte-adjacent -> packed layout produced by the
    2-byte hardware DMA-transpose of fp8 pairs inside bf16 elements)
  - the grouped-vs-packed pair mapping is reconciled by the host column
    interleave of w; the [OUT, S] device output is transposed back on
    the host during the gather
  - engine/queue discipline, chosen from NTFF profile analysis: DVE runs
    ONLY x-binarize + psum-evict (x-binarize for chain sc+2 is emitted
    at the top of iteration sc, ahead of that iteration's evictions, so
    the strict-FIFO DVE queue never head-of-line-blocks the x pipeline
    behind evictions that wait on matmuls); ACT runs all w-binarize
    (prologue-only); sync HWDGE = x/w loads, scalar HWDGE =
    DMA-transposes only, gpsimd SWDGE = late w loads + output stores
  - result is bit-exact (all products +/-0.5, exact fp32 accumulation)
"""

import sys

if "/opt/trn_rl_repo" not in sys.path:
    sys.path.insert(0, "/opt/trn_rl_repo")

import numpy as np

B, S, IN, OUT = 4, 2048, 4096, 16384
N_CORES = 8
O_SHARD = OUT // N_CORES  # 2048
P = 128  # partitions
N_CH = 512  # psum free-dim chunk (one bank of fp32)

# dev knobs (test.py only; harness uses defaults)
_TRACE = False
_LAST_RESULTS = None


def build_program(s_rows=B * S, o_shard=O_SHARD, kdim=IN, scale=1.0,
                  zero_thr=True):
    """Trace the single-core SPMD program.

    Inputs: x [s_rows,kdim] bf16, w [kdim,o_shard] bf16 (host-interleaved
    columns then transposed), thr [o_shard,1] f32.
    Output: outT [o_shard,s_rows] f32.
    `scale` is the full eviction scale (pow2 shift x binarize-value
    compensation), computed on host.
    """
    import concourse.bass as bass
    import concourse.mybir as mybir
    import concourse.tile as tile
    from concourse import bacc
    from concourse.alu_op_type import AluOpType

    f32 = mybir.dt.float32
    bf16 = mybir.dt.bfloat16
    fp8 = mybir.dt.float8e4
    Sign = mybir.ActivationFunctionType.Sign

    n_g = kdim // 256      # DoubleRow groups (256 contraction rows each)
    n_kt = kdim // P       # 128-row k-tiles in the stationary slab
    n_ob = o_shard // P    # o-blocks of 128
    n_pass = n_ob // 4     # 4 o-blocks (psum banks) per pass
    n_sc = s_rows // N_CH  # s-chunks of 512
    MC = min(4, n_kt)      # k-tiles per w load chunk
    n_mc = n_kt // MC
    assert s_rows % N_CH == 0 and o_shard % (4 * P) == 0 and kdim % 256 == 0
    assert n_kt % MC == 0 and n_sc >= 4

    nc = bacc.Bacc(None, target_bir_lowering=False, debug=False)

    w_dt = bf16 if zero_thr else f32
    x_d = nc.dram_tensor("x", [s_rows, kdim], bf16, kind="ExternalInput")
    w_d = nc.dram_tensor("w", [kdim, o_shard], w_dt, kind="ExternalInput")
    t_d = nc.dram_tensor("thr", [o_shard, 1], f32, kind="ExternalInput")
    o_d = nc.dram_tensor("outT", [o_shard, s_rows], f32, kind="ExternalOutput")

    with tile.TileContext(nc) as tc:
        with (
            tc.tile_pool(name="raw", bufs=3) as raw_pool,
            tc.tile_pool(name="wld", bufs=4) as wld_pool,
            tc.tile_pool(name="b8", bufs=5) as b8_pool,
            tc.tile_pool(name="xt", bufs=3) as xt_pool,
            tc.tile_pool(name="w8", bufs=1) as w8_pool,
            tc.tile_pool(name="outp", bufs=3) as out_pool,
            tc.tile_pool(name="misc", bufs=1) as misc_pool,
            tc.tile_pool(name="ps", bufs=2, space="PSUM") as ps_pool,
        ):
            # Binarized x values live as fp8 (+/-0.5). Two fp8 values for
            # consecutive (interleaved) contraction rows pack into one
            # bf16-typed element so the 2-byte hardware DMA-transpose moves
            # them in one shot; the pair becomes DoubleRow's two k-groups
            # via a bitcast AP.

            wslabs = [
                w8_pool.tile([P, n_kt, 4 * P], fp8, name=f"wslab{ps}",
                             tag=f"wslab{ps}")
                for ps in range(n_pass)
            ]

            thr_rep = None
            if not zero_thr:
                # broadcast thr [o_shard] across partitions via a rank-1
                # matmul: ones[1,128].T @ thr_row[1, o] -> [128, o]
                thr_rep = misc_pool.tile([P, o_shard], f32, name="thr_rep")
                ones_t = misc_pool.tile([P, P], f32, name="ones_t")
                thr_row = misc_pool.tile([P, o_shard], f32, name="thr_row")
                nc.vector.memset(ones_t[:], 1.0)
                nc.sync.dma_start(thr_row[:1, :],
                                  t_d[:, :].rearrange("o one -> one o"))
                for q in range(o_shard // N_CH):
                    tps = ps_pool.tile([P, N_CH], f32, name="tps", tag="ps0")
                    nc.tensor.matmul(tps[:], ones_t[:1, :P],
                                     thr_row[:1, q * N_CH:(q + 1) * N_CH],
                                     start=True, stop=True)
                    nc.vector.tensor_copy(
                        thr_rep[:, q * N_CH:(q + 1) * N_CH], tps[:])

            def w_dma(ps, mc, eng=None):
                # one DMA pulls MC k-tiles of this pass's o-range into
                # [p, t, o] layout straight from the host-transposed wT
                wtile = wld_pool.tile([P, MC, 4 * P], w_dt, name="wtile",
                                      tag="wld")
                src = w_d[mc * MC * P:(mc + 1) * MC * P,
                          ps * 4 * P:(ps + 1) * 4 * P]
                (eng or nc.sync).dma_start(
                    wtile[:], src.rearrange("(t p) o -> p t o", p=P))
                return wtile

            def w_bin(ps, mc, wtile):
                dst = wslabs[ps][:, mc * MC:(mc + 1) * MC, :]
                if zero_thr:
                    # Sign(w) -> +/-1 on ACT, keeping DVE free for the x
                    # pipeline; exact for all non-zero w (the host routes
                    # any input containing an exact zero to the general
                    # path, where is_ge handles it)
                    nc.scalar.activation(dst, wtile[:], Sign)
                else:
                    for t in range(MC):
                        sel = thr_rep[:, ps * 4 * P:(ps + 1) * 4 * P]
                        nc.vector.scalar_tensor_tensor(
                            dst[:, t, :], wtile[:, t, :], 1.0, sel,
                            op0=AluOpType.mult, op1=AluOpType.is_ge)
                        nc.vector.tensor_scalar(
                            dst[:, t, :], dst[:, t, :], 0.5, None,
                            AluOpType.subtract)

            def prep_chunk(ps, mc, eng=None):
                w_bin(ps, mc, w_dma(ps, mc, eng))

            def chain_raws(sc):
                raws = []
                for sub in range(4):
                    s0 = sc * N_CH + sub * P
                    x_raw = raw_pool.tile([P, kdim], bf16, name="x_raw",
                                          tag="raw")
                    nc.sync.dma_start(x_raw[:], x_d[s0:s0 + P, :])
                    raws.append(x_raw)
                return raws

            def chain_finish(sc, raws):
                # x moving tile [p, g, 512 s] as packed fp8 pairs in bf16:
                # filled by 4 DMA-transposes (one per 128-row s-subblock)
                xtile = xt_pool.tile([P, n_g, N_CH], bf16, name="xtile",
                                     tag="xt")
                for sub in range(4):
                    xb8 = b8_pool.tile([P, kdim], fp8, name="xb8", tag="b8")
                    nc.vector.tensor_scalar(
                        xb8[:], raws[sub][:], 0.0, 0.5,
                        AluOpType.is_ge, AluOpType.subtract)
                    nc.scalar.dma_start(
                        xtile[:, :, sub * P:(sub + 1) * P],
                        xb8[:].bitcast(bf16), transpose=True)
                return xtile.bitcast(fp8)  # [p, g, 1024] (s,j interleaved)

            def emit_chain(sc):
                return chain_finish(sc, chain_raws(sc))

            # --- prologue ---
            # demand-ordered: chain-0 x loads first, then pass-0 w chunks,
            # then chain 1 / pass 1 (all on the sync queue); passes 2-3
            # load over the idle SWDGE path.  All w-binarize lands on ACT,
            # all x-binarize on DVE, so neither pipeline queues behind the
            # other.
            chains = {}
            raws0 = chain_raws(0)
            wt0 = [w_dma(0, mc) for mc in range(min(2, n_mc))]
            chains[0] = chain_finish(0, raws0)
            for mc, wt in enumerate(wt0):
                w_bin(0, mc, wt)
            for mc in range(2, n_mc):
                prep_chunk(0, mc)
            if n_sc >= 2:
                chains[1] = emit_chain(1)
            if n_pass >= 2:
                for mc in range(n_mc):
                    prep_chunk(1, mc)
            for ps in range(2, n_pass):
                for mc in range(n_mc):
                    prep_chunk(ps, mc, nc.gpsimd)

            # --- main loop over s-chunks of 512 ---
            for sc in range(n_sc):
                # emit the sc+2 chain ahead of this iteration's evictions:
                # its DVE binarizes clear the queue before any eviction
                # (which waits on matmuls) can block them
                if sc + 2 < n_sc:
                    chains[sc + 2] = emit_chain(sc + 2)
                xt8 = chains.pop(sc)

                for ps in range(n_pass):
                    pss = [
                        ps_pool.tile([P, N_CH], f32, name=f"ps{i}",
                                     tag=f"ps{i}")
                        for i in range(4)
                    ]
                    for g in range(n_g):
                        rhs = xt8[:, g, :].rearrange("p (s j) -> p j s", j=2)
                        for i in range(4):
                            nc.tensor.matmul(
                                pss[i][:],
                                wslabs[ps][:, 2 * g:2 * g + 2,
                                           i * P:(i + 1) * P],
                                rhs,
                                start=(g == 0), stop=(g == n_g - 1),
                                perf_mode=mybir.MatmulPerfMode.DoubleRow)
                    for i in range(4):
                        ob = ps * 4 + i
                        ot = out_pool.tile([P, N_CH], f32, name="ot", tag="ot")
                        # psum eviction with the pow2 scale folded in
                        nc.vector.tensor_scalar(
                            ot[:], pss[i][:], float(scale), None,
                            AluOpType.mult)
                        nc.gpsimd.dma_start(
                            o_d[ob * P:(ob + 1) * P,
                                sc * N_CH:(sc + 1) * N_CH], ot[:])

    nc.compile()
    return nc


def _host_scale(shift_param, zero_thr):
    # undoes the binarize-value factors (x: +/-0.5 always; w: +/-1 when
    # zero_thr -> x2, +/-0.5 otherwise -> x4);
    # np.round is round-half-to-even, matching jnp.round.
    s = np.clip(np.float64(np.float32(shift_param)), -8.0, 0.0)
    comp = 2.0 if zero_thr else 4.0
    return comp * float(np.exp2(np.round(s)))


def _interleave_w_cols(w):
    """Host permutation so the device's grouped stationary layout pairs the
    same contraction rows as the packed moving layout: new col 256g+128j+p
    holds old col 256g+2p+j."""
    o, k = w.shape
    return np.ascontiguousarray(
        w.reshape(o, k // 256, 128, 2).transpose(0, 1, 3, 2).reshape(o, k))


def kernel(x, weight, threshold, shift_param):
    import ml_dtypes

    from concourse.bass_utils import run_bass_kernel_spmd

    bf16 = ml_dtypes.bfloat16
    thr_f = np.asarray(threshold, np.float32).reshape(OUT, 1)
    w_f = weight.astype(np.float32)
    # the fast path binarizes w with Sign on-device; exact zeros (which
    # Sign maps to 0, not +1) go to the general threshold path instead
    zero_thr = bool(np.all(thr_f == 0.0)) and not bool(np.any(w_f == 0.0))
    scale = _host_scale(shift_param, zero_thr)
    nc = build_program(scale=scale, zero_thr=zero_thr)

    xf = np.ascontiguousarray(
        x.astype(np.float32).reshape(B * S, IN).astype(bf16))
    # layout-only host prep of w: column interleave, transpose; the cast
    # to bf16 is sign-exact and only taken when threshold is all-zero
    wt = _interleave_w_cols(w_f)
    wt = (wt.astype(bf16) if zero_thr else wt).T
    in_maps = []
    for c in range(N_CORES):
        sl = slice(c * O_SHARD, (c + 1) * O_SHARD)
        in_maps.append({
            "x": xf,
            "w": np.ascontiguousarray(wt[:, sl]),
            "thr": np.ascontiguousarray(thr_f[sl]),
        })

    res = run_bass_kernel_spmd(nc, in_maps, list(range(N_CORES)), trace=_TRACE)
    global _LAST_RESULTS
    _LAST_RESULTS = res
    shards = [res.results[c]["outT"] for c in range(N_CORES)]
    full_t = np.concatenate(shards, axis=0)  # [OUT, B*S]
    full = np.ascontiguousarray(full_t.T).reshape(B, S, OUT)
    return full.astype(np.float32)
